# revision 12
# baseline (speedup 1.0000x reference)
"""Expert-parallel MoE (top-2 of 8 experts, SwiGLU FFN) for 8 Trainium2 cores.

v2 design (see git history for the f32r baseline):
  - Host computes the small gate (logits -> top-2 -> softmax) in float64
    numpy and dispatches tokens to experts.
  - bf16 matmuls everywhere: the PE runs bf16 at a clean 1 row/cycle with
    fast-weight-load, avoiding the ~9%/column penalty measured on the
    f32r path. End-to-end rel err vs the f32 reference is ~4e-3, well
    inside the 2e-2 gate.
  - Load balancing: capacity per core is nA*a + nB*b tokens, laid out as
    nA "main" chunks (weight set 0 = the core's own expert) plus nB small
    "overflow" chunks (weight sets 1..nB, fed by the host with whichever
    heavy expert overflowed). Since weights are per-core input data, any
    (expert -> chunk-slot) assignment satisfies SPMD. A tiny runtime
    solver picks (nA, a, nB, b) from the actual expert counts, so the
    per-core capacity is ~2068 instead of max-expert-count ~2152.
  - Single weight pass: x and h stay SBUF-resident for the whole kernel;
    every weight tile is DMA'd exactly once (~35MB/core with 2 sets),
    which removes the per-group weight re-streaming stalls the baseline
    paid for. Chunks are processed A0..A(nA-1) then the B runts, so the
    kernel tail is one small chunk's drain.

Device program per core (feature-major: features on partitions, tokens
on the free axis, so h feeds the down-projection without transposes):
  UP:   for ht in 22:  h[ht, c] = silu(x @ Wg[ht]) * (x @ Wu[ht])  per chunk c
  DOWN: for dt in 8:   y[dt, c] = h @ Wd[dt]                       per chunk c
"""

import numpy as np

DIM = 1024
HID = 2816
E = 8
TOPK = 2
P = 128
KD = DIM // P   # 8 k-subtiles (contraction of x@W)
HT = HID // P   # 22 h-subtiles
DT = DIM // P   # 8 d-subtiles (output features)

CHUNK_MAX = 512         # PSUM bank limit (512 fp32 accumulators)

# bisection flags (perf experiments; both False = v2 baseline)
X_SPLIT = True   # x as per-chunk tiles instead of one [P, KD, C] tile
H_SPLIT = False  # h as per-chunk tiles instead of one [P, HT, C] tile

_KERNEL_CACHE = {}
LAST_RESULTS = None  # BassKernelResults of the most recent run (for profiling)


def _solve_profile(counts):
    """Pick (nA, a, nB, b): each core runs nA main chunks of width a for
    its own expert, plus nB overflow chunks of width b whose expert the
    host chooses per-core. Feasibility: every expert fits its first nA*a
    tokens in its main slot, and the overflow (count - nA*a) of all
    experts packs into the 8*nB b-wide slots (each slot one expert).
    Minimize the modeled tensor-engine time: each chunk of width w costs
    max(w, OVH) + ISS columns (OVH ~ the per-matmul LDWEIGHTS/issue floor
    at small N, ISS ~ per-matmul NX issue overhead), so tiny runt chunks
    and excessive chunk counts are both charged."""
    OVH, ISS = 130, 8

    def chunk_cost(w):
        return max(w, OVH) + ISS
    best = None
    cmax = max(counts)
    for nA in range(4, 18):
        a_lo = max(2, (cmax + nA - 1) // nA - 256)
        for a in range(CHUNK_MAX, a_lo - 1, -2):
            acap = nA * a
            deficits = [c - acap for c in counts if c > acap]
            if not deficits:
                cand = (nA * chunk_cost(a), nA + 0, nA, a, 0, 0)
                if best is None or (cand[0], cand[1]) < (best[0], best[1]):
                    best = cand
                continue
            for nB in (1, 2):
                slots = 8 * nB
                if len(deficits) > slots:
                    continue
                # minimal even b with sum(ceil(d/b)) <= slots
                lo, hi = 2, CHUNK_MAX
                bb = None
                while lo <= hi:
                    mid = ((lo + hi) // 2) & ~1
                    if mid < lo:
                        mid = lo
                    need = sum((d + mid - 1) // mid for d in deficits)
                    if need <= slots:
                        bb = mid
                        hi = mid - 2
                    else:
                        lo = mid + 2
                if bb is None:
                    continue
                cost = nA * chunk_cost(a) + nB * chunk_cost(bb)
                cand = (cost, nA + nB, nA, a, nB, bb)
                if best is None or (cand[0], cand[1]) < (best[0], best[1]):
                    best = cand
    assert best is not None, f"no feasible profile for counts {counts}"
    _, _, nA, a, nB, b = best
    return nA, a, nB, b


def _build_moe_ffn(nA, a, nB, b):
    """Per-core Bass program: y^T = SwiGLU FFN of x^T, both feature-major.
    Chunks 0..nA-1 (width a) use weight set 0; chunk nA+j (width b) uses
    weight set 1+j."""
    import concourse.bass as bass  # noqa: F401
    import concourse.mybir as mybir
    from concourse import bacc, tile

    f32 = mybir.dt.float32
    bf16 = mybir.dt.bfloat16
    SiLU = mybir.ActivationFunctionType.Silu

    C = nA * a + nB * b
    NSET = 1 + nB
    # (offset, width, weight-set) per chunk; runts last so the kernel tail
    # is a small chunk's drain
    chunks = [(i * a, a, 0) for i in range(nA)]
    chunks += [(nA * a + j * b, b, 1 + j) for j in range(nB)]

    nc = bacc.Bacc("TRN2", target_bir_lowering=False, debug=False)

    xt = nc.dram_tensor("xt", [P, KD, C], bf16, kind="ExternalInput")
    wgt = nc.dram_tensor("wgt", [NSET, HT, P, KD, P], bf16, kind="ExternalInput")
    wut = nc.dram_tensor("wut", [NSET, HT, P, KD, P], bf16, kind="ExternalInput")
    wdt = nc.dram_tensor("wdt", [NSET, DT, P, HT, P], bf16, kind="ExternalInput")
    yt = nc.dram_tensor("yt", [DT, P, C], f32, kind="ExternalOutput")

    with tile.TileContext(nc) as tc:
        with (
            tc.tile_pool(name="xp", bufs=1) as xp,
            tc.tile_pool(name="hp", bufs=1) as hp,
            tc.tile_pool(name="wp", bufs=2) as wp,
            tc.tile_pool(name="dp", bufs=2) as dp,
            tc.tile_pool(name="op", bufs=3) as op,
            tc.tile_pool(name="ps", bufs=1, space="PSUM") as ps,
        ):
            # ---- persistent SBUF tensors ----
            if H_SPLIT:
                h_tiles = [
                    hp.tile([P, HT, w], bf16, tag=f"h{ci}", name=f"h{ci}")
                    for ci, (off, w, _s) in enumerate(chunks)
                ]

                def h_view(ci, ht, off, w):
                    return h_tiles[ci][:, ht]
            else:
                h_sb = hp.tile([P, HT, C], bf16, tag="h")

                def h_view(ci, ht, off, w):
                    return h_sb[:, ht, off : off + w]

            def load_w(ht):
                # one [P, KD*P] tile per (kind, set); set 0 first (the
                # opening matmuls need it before the overflow sets)
                tiles = []
                for s in range(NSET):
                    g = wp.tile([P, KD, P], bf16, tag=f"wg{s}", name=f"wg{s}_{ht}")
                    nc.sync.dma_start(g[:], wgt[s, ht])
                    u = wp.tile([P, KD, P], bf16, tag=f"wu{s}", name=f"wu{s}_{ht}")
                    nc.sync.dma_start(u[:], wut[s, ht])
                    tiles.append((g, u))
                return tiles

            # x per-chunk tiles; chunk 0's x is the first DMA issued (the
            # opening sub-phase only needs it plus one weight tile)
            assert X_SPLIT and not H_SPLIT
            x_tiles = [None] * len(chunks)

            def load_x(ci):
                off, w, _s = chunks[ci]
                xc = xp.tile([P, KD, w], bf16, tag=f"x{ci}", name=f"x{ci}")
                nc.sync.dma_start(xc[:], xt[:, :, off : off + w])
                x_tiles[ci] = xc

            def x_view(ci, kt, off, w):
                return x_tiles[ci][:, kt]

            load_x(0)

            def up_chunk(ci, ht, wg_sb, wu_sb):
                off, w, s = chunks[ci]
                pg = ps.tile([P, w], f32, tag="pg", bufs=3)
                pu = ps.tile([P, w], f32, tag="pu", bufs=3)
                for kt in range(KD):
                    nc.tensor.matmul(
                        pg, wg_sb[:, kt], x_view(ci, kt, off, w),
                        start=(kt == 0), stop=(kt == KD - 1),
                    )
                for kt in range(KD):
                    nc.tensor.matmul(
                        pu, wu_sb[:, kt], x_view(ci, kt, off, w),
                        start=(kt == 0), stop=(kt == KD - 1),
                    )
                sl = op.tile([P, w], f32, tag="silu")
                nc.scalar.activation(sl[:], pg, SiLU)
                nc.vector.tensor_mul(h_view(ci, ht, off, w), sl[:], pu)

            # ---- UP phase 0: chunk 0 alone, ht-inner, set-0 weights
            # streamed through a small ring. Starts computing after ~1.2MB
            # of DMA instead of waiting for all of x. ----
            w_cache = {}
            p0_cache = {}

            def load_w0(ht):
                g = wp.tile([P, KD, P], bf16, tag="p0g", name=f"p0g_{ht}")
                nc.sync.dma_start(g[:], wgt[0, ht])
                u = wp.tile([P, KD, P], bf16, tag="p0u", name=f"p0u_{ht}")
                nc.sync.dma_start(u[:], wut[0, ht])
                p0_cache[ht] = (g, u)

            load_w0(0)
            for ht in range(HT):
                if ht not in p0_cache:
                    load_w0(ht)
                if ht + 1 < HT:
                    load_w0(ht + 1)
                # spread the remaining x prefetches between weight loads so
                # they don't starve the phase-0 weight stream
                if ht % 4 == 3 and (ht // 4) + 1 < len(chunks):
                    load_x((ht // 4) + 1)
                if ht == HT - 4:
                    # phase 1's first weights (both sets)
                    w_cache[0] = load_w(0)
                g, u = p0_cache.pop(ht)
                up_chunk(0, ht, g, u)

            for ci in range(len(chunks)):
                if x_tiles[ci] is None:
                    load_x(ci)

            # ---- UP phase 1: remaining chunks, ht-outer (weights once) ----
            for ht in range(HT):
                if ht not in w_cache:
                    w_cache[ht] = load_w(ht)
                if ht + 1 < HT:
                    w_cache[ht + 1] = load_w(ht + 1)
                sets = w_cache.pop(ht)
                for ci, (off, w, s) in enumerate(chunks):
                    if ci == 0:
                        continue
                    wg_sb, wu_sb = sets[s]
                    up_chunk(ci, ht, wg_sb, wu_sb)

            # ---- DOWN: y = h @ Wd, feature-major [DIM, C] ----
            def load_wd(dt):
                tiles = []
                for s in range(NSET):
                    d = dp.tile([P, HT, P], bf16, tag=f"wd{s}", name=f"wd{s}_{dt}")
                    nc.sync.dma_start(d[:], wdt[s, dt])
                    tiles.append(d)
                return tiles

            d_cache = {0: load_wd(0)}
            for dt in range(DT):
                if dt not in d_cache:
                    d_cache[dt] = load_wd(dt)
                if dt + 1 < DT:
                    d_cache[dt + 1] = load_wd(dt + 1)
                sets = d_cache.pop(dt)
                for ci, (off, w, s) in enumerate(chunks):
                    wd_sb = sets[s]
                    py = ps.tile([P, w], f32, tag="py", bufs=2)
                    for ht in range(HT):
                        nc.tensor.matmul(
                            py, wd_sb[:, ht], h_view(ci, ht, off, w),
                            start=(ht == 0), stop=(ht == HT - 1),
                        )
                    o_sb = op.tile([P, w], f32, tag="o")
                    nc.vector.tensor_copy(o_sb[:], py)
                    nc.sync.dma_start(yt[dt, :, off : off + w], o_sb[:])

    nc.finalize()
    return nc


def _get_kernel(nA, a, nB, b):
    key = (nA, a, nB, b)
    if key not in _KERNEL_CACHE:
        _KERNEL_CACHE[key] = _build_moe_ffn(nA, a, nB, b)
    return _KERNEL_CACHE[key]


def _route(xf, W_gate):
    """Replicate reference routing: top-2 by logit, softmax weights.

    float64 logits: the top-k decision boundary gap is >> f32 rounding
    noise, so this matches the f32 jax reference's selection."""
    logits = xf.astype(np.float64) @ W_gate.astype(np.float64)  # [N, E]
    order = np.argsort(-logits, axis=1, kind="stable")[:, :TOPK]  # [N, 2]
    top = np.take_along_axis(logits, order, axis=1)
    top = top - top.max(axis=1, keepdims=True)
    ew = np.exp(top)
    w = (ew / ew.sum(axis=1, keepdims=True)).astype(np.float32)  # [N, 2]
    return order, w


def _to_bf16(arr):
    import ml_dtypes

    return np.ascontiguousarray(arr.astype(ml_dtypes.bfloat16))


def kernel(x, W_gate, Wg, Wu, Wd):
    from concourse.bass_utils import run_bass_kernel_spmd

    x = np.ascontiguousarray(np.asarray(x, dtype=np.float32))
    W_gate = np.asarray(W_gate, dtype=np.float32)
    Wg = np.asarray(Wg, dtype=np.float32)
    Wu = np.asarray(Wu, dtype=np.float32)
    Wd = np.asarray(Wd, dtype=np.float32)

    B, T, D = x.shape
    xf = x.reshape(-1, D)
    N = xf.shape[0]

    order, w = _route(xf, W_gate)

    ids = []  # per-expert token indices
    wts = []  # per-expert combine weights
    for e in range(E):
        sel = np.nonzero(order == e)
        ids.append(sel[0])
        wts.append(w[sel[0], sel[1]])
    counts = [len(i) for i in ids]

    nA, a, nB, b = _solve_profile(counts)
    acap = nA * a
    C = acap + nB * b
    nc = _get_kernel(nA, a, nB, b)

    # ---- assign overflow (beyond each expert's main slot) to B-slots ----
    # slots[core][j] = (expert, token_ids, token_wts) or None
    slots = [[None] * nB for _ in range(E)]
    free = [(core, j) for j in range(nB) for core in range(E)]
    overflow = []  # (size, expert, ids, wts) slices of width <= b
    for e in range(E):
        rem_i = ids[e][acap:]
        rem_w = wts[e][acap:]
        for s0 in range(0, len(rem_i), b):
            overflow.append((e, rem_i[s0 : s0 + b], rem_w[s0 : s0 + b]))
    assert len(overflow) <= len(free), (counts, nA, a, nB, b)
    for (e, oi, ow), (core, j) in zip(overflow, free):
        slots[core][j] = (e, oi, ow)

    # ---- weight layout transforms (bf16, feature-major tiles) ----
    def wg_tiles(e):
        return Wg[e].reshape(KD, P, HT, P).transpose(2, 1, 0, 3)

    def wu_tiles(e):
        return Wu[e].reshape(KD, P, HT, P).transpose(2, 1, 0, 3)

    def wd_tiles(e):
        return Wd[e].reshape(HT, P, DT, P).transpose(2, 1, 0, 3)

    in_maps = []
    for core in range(E):
        xe = np.zeros((C, DIM), dtype=np.float32)
        cnt_main = min(counts[core], acap)
        xe[:cnt_main] = xf[ids[core][:cnt_main]]
        wg_s = np.zeros((1 + nB, HT, P, KD, P), dtype=np.float32)
        wu_s = np.zeros_like(wg_s)
        wd_s = np.zeros((1 + nB, DT, P, HT, P), dtype=np.float32)
        wg_s[0] = wg_tiles(core)
        wu_s[0] = wu_tiles(core)
        wd_s[0] = wd_tiles(core)
        for j in range(nB):
            if slots[core][j] is None:
                continue
            e, oi, _ow = slots[core][j]
            xe[acap + j * b : acap + j * b + len(oi)] = xf[oi]
            wg_s[1 + j] = wg_tiles(e)
            wu_s[1 + j] = wu_tiles(e)
            wd_s[1 + j] = wd_tiles(e)
        x_t = _to_bf16(xe.T.reshape(KD, P, C).transpose(1, 0, 2))
        in_maps.append(
            {
                "xt": x_t,
                "wgt": _to_bf16(wg_s),
                "wut": _to_bf16(wu_s),
                "wdt": _to_bf16(wd_s),
            }
        )

    res = run_bass_kernel_spmd(nc, in_maps, core_ids=list(range(E)))
    global LAST_RESULTS
    LAST_RESULTS = res

    out = np.zeros((N, D), dtype=np.float32)
    for core in range(E):
        y = res.results[core]["yt"].reshape(DIM, C)  # feature-major
        cnt_main = min(counts[core], acap)
        out[ids[core][:cnt_main]] += (
            wts[core][:cnt_main][:, None] * y[:, :cnt_main].T
        )
        for j in range(nB):
            if slots[core][j] is None:
                continue
            _e, oi, ow = slots[core][j]
            lo = acap + j * b
            out[oi] += ow[:, None] * y[:, lo : lo + len(oi)].T
    return out.reshape(B, T, D)


# revision 13
# speedup vs baseline: 1.1926x; 1.1926x over previous
"""Expert-parallel MoE (top-2 of 8 experts, SwiGLU FFN) for 8 Trainium2 cores.

v2 design (see git history for the f32r baseline):
  - Host computes the small gate (logits -> top-2 -> softmax) in float64
    numpy and dispatches tokens to experts.
  - bf16 matmuls everywhere: the PE runs bf16 at a clean 1 row/cycle with
    fast-weight-load, avoiding the ~9%/column penalty measured on the
    f32r path. End-to-end rel err vs the f32 reference is ~4e-3, well
    inside the 2e-2 gate.
  - Load balancing: capacity per core is nA*a + nB*b tokens, laid out as
    nA "main" chunks (weight set 0 = the core's own expert) plus nB small
    "overflow" chunks (weight sets 1..nB, fed by the host with whichever
    heavy expert overflowed). Since weights are per-core input data, any
    (expert -> chunk-slot) assignment satisfies SPMD. A tiny runtime
    solver picks (nA, a, nB, b) from the actual expert counts, so the
    per-core capacity is ~2068 instead of max-expert-count ~2152.
  - Single weight pass: x and h stay SBUF-resident for the whole kernel;
    every weight tile is DMA'd exactly once (~35MB/core with 2 sets),
    which removes the per-group weight re-streaming stalls the baseline
    paid for. Chunks are processed A0..A(nA-1) then the B runts, so the
    kernel tail is one small chunk's drain.

Device program per core (feature-major: features on partitions, tokens
on the free axis, so h feeds the down-projection without transposes):
  UP:   for ht in 22:  h[ht, c] = silu(x @ Wg[ht]) * (x @ Wu[ht])  per chunk c
  DOWN: for dt in 8:   y[dt, c] = h @ Wd[dt]                       per chunk c
"""

import numpy as np

DIM = 1024
HID = 2816
E = 8
TOPK = 2
P = 128
KD = DIM // P   # 8 k-subtiles (contraction of x@W)
HT = HID // P   # 22 h-subtiles
DT = DIM // P   # 8 d-subtiles (output features)

CHUNK_MAX = 512         # PSUM bank limit (512 fp32 accumulators)

# bisection flags (perf experiments; both False = v2 baseline)
X_SPLIT = True   # x as per-chunk tiles instead of one [P, KD, C] tile
H_SPLIT = False  # h as per-chunk tiles instead of one [P, HT, C] tile

_KERNEL_CACHE = {}
LAST_RESULTS = None  # BassKernelResults of the most recent run (for profiling)


def _solve_profile(counts):
    """Pick (nA, a, nB, b): each core runs nA main chunks of width a for
    its own expert, plus nB overflow chunks of width b whose expert the
    host chooses per-core. Feasibility: every expert fits its first nA*a
    tokens in its main slot, and the overflow (count - nA*a) of all
    experts packs into the 8*nB b-wide slots (each slot one expert).
    Minimize the modeled tensor-engine time: each chunk of width w costs
    max(w, OVH) + ISS columns (OVH ~ the per-matmul LDWEIGHTS/issue floor
    at small N, ISS ~ per-matmul NX issue overhead), so tiny runt chunks
    and excessive chunk counts are both charged."""
    OVH, ISS = 130, 8

    def chunk_cost(w):
        return max(w, OVH) + ISS
    best = None
    cmax = max(counts)
    for nA in range(4, 18):
        a_lo = max(2, (cmax + nA - 1) // nA - 256)
        for a in range(CHUNK_MAX, a_lo - 1, -2):
            acap = nA * a
            deficits = [c - acap for c in counts if c > acap]
            if not deficits:
                cand = (nA * chunk_cost(a), nA + 0, nA, a, 0, 0)
                if best is None or (cand[0], cand[1]) < (best[0], best[1]):
                    best = cand
                continue
            for nB in (1, 2):
                slots = 8 * nB
                if len(deficits) > slots:
                    continue
                # minimal even b with sum(ceil(d/b)) <= slots
                lo, hi = 2, CHUNK_MAX
                bb = None
                while lo <= hi:
                    mid = ((lo + hi) // 2) & ~1
                    if mid < lo:
                        mid = lo
                    need = sum((d + mid - 1) // mid for d in deficits)
                    if need <= slots:
                        bb = mid
                        hi = mid - 2
                    else:
                        lo = mid + 2
                if bb is None:
                    continue
                cost = nA * chunk_cost(a) + nB * chunk_cost(bb)
                cand = (cost, nA + nB, nA, a, nB, bb)
                if best is None or (cand[0], cand[1]) < (best[0], best[1]):
                    best = cand
    assert best is not None, f"no feasible profile for counts {counts}"
    _, _, nA, a, nB, b = best
    return nA, a, nB, b


def _build_moe_ffn(nA, a, nB, b):
    """Per-core Bass program: y^T = SwiGLU FFN of x^T, both feature-major.
    Chunks 0..nA-1 (width a) use weight set 0; chunk nA+j (width b) uses
    weight set 1+j."""
    import concourse.bass as bass  # noqa: F401
    import concourse.mybir as mybir
    from concourse import bacc, tile

    f32 = mybir.dt.float32
    bf16 = mybir.dt.bfloat16
    SiLU = mybir.ActivationFunctionType.Silu

    C = nA * a + nB * b
    NSET = 1 + nB
    # (offset, width, weight-set) per chunk; runts last so the kernel tail
    # is a small chunk's drain
    chunks = [(i * a, a, 0) for i in range(nA)]
    chunks += [(nA * a + j * b, b, 1 + j) for j in range(nB)]

    nc = bacc.Bacc("TRN2", target_bir_lowering=False, debug=False)

    xt = nc.dram_tensor("xt", [P, KD, C], bf16, kind="ExternalInput")
    wgt = nc.dram_tensor("wgt", [NSET, HT, P, KD, P], bf16, kind="ExternalInput")
    wut = nc.dram_tensor("wut", [NSET, HT, P, KD, P], bf16, kind="ExternalInput")
    wdt = nc.dram_tensor("wdt", [NSET, DT, P, HT, P], bf16, kind="ExternalInput")
    yt = nc.dram_tensor("yt", [DT, P, C], f32, kind="ExternalOutput")

    with tile.TileContext(nc) as tc:
        with (
            tc.tile_pool(name="xp", bufs=1) as xp,
            tc.tile_pool(name="hp", bufs=1) as hp,
            tc.tile_pool(name="wp", bufs=2) as wp,
            tc.tile_pool(name="dp", bufs=2) as dp,
            tc.tile_pool(name="op", bufs=3) as op,
            tc.tile_pool(name="ps", bufs=1, space="PSUM") as ps,
        ):
            # ---- persistent SBUF tensors ----
            if H_SPLIT:
                h_tiles = [
                    hp.tile([P, HT, w], bf16, tag=f"h{ci}", name=f"h{ci}")
                    for ci, (off, w, _s) in enumerate(chunks)
                ]

                def h_view(ci, ht, off, w):
                    return h_tiles[ci][:, ht]
            else:
                h_sb = hp.tile([P, HT, C], bf16, tag="h")

                def h_view(ci, ht, off, w):
                    return h_sb[:, ht, off : off + w]

            def load_w(ht):
                # one [P, KD*P] tile per (kind, set); set 0 first (the
                # opening matmuls need it before the overflow sets)
                tiles = []
                for s in range(NSET):
                    g = wp.tile([P, KD, P], bf16, tag=f"wg{s}", name=f"wg{s}_{ht}")
                    nc.sync.dma_start(g[:], wgt[s, ht])
                    u = wp.tile([P, KD, P], bf16, tag=f"wu{s}", name=f"wu{s}_{ht}")
                    nc.sync.dma_start(u[:], wut[s, ht])
                    tiles.append((g, u))
                return tiles

            # x per-chunk tiles (keeping the moving operand in small
            # dedicated tiles is what keeps the PE at full clock — one big
            # multi-DMA'd x tile measured ~20% slower, see history)
            assert X_SPLIT and not H_SPLIT
            w_cache = {0: load_w(0)}
            x_tiles = []
            for ci, (off, w, _s) in enumerate(chunks):
                xc = xp.tile([P, KD, w], bf16, tag=f"x{ci}", name=f"x{ci}")
                nc.sync.dma_start(xc[:], xt[:, :, off : off + w])
                x_tiles.append(xc)

            def x_view(ci, kt, off, w):
                return x_tiles[ci][:, kt]

            # ---- UP: h = silu(x@Wg) * (x@Wu), ht-outer (weights once) ----
            for ht in range(HT):
                if ht not in w_cache:
                    w_cache[ht] = load_w(ht)
                if ht + 1 < HT:
                    w_cache[ht + 1] = load_w(ht + 1)
                sets = w_cache.pop(ht)
                for ci, (off, w, s) in enumerate(chunks):
                    wg_sb, wu_sb = sets[s]
                    pg = ps.tile([P, w], f32, tag="pg", bufs=3)
                    pu = ps.tile([P, w], f32, tag="pu", bufs=3)
                    for kt in range(KD):
                        nc.tensor.matmul(
                            pg, wg_sb[:, kt], x_view(ci, kt, off, w),
                            start=(kt == 0), stop=(kt == KD - 1),
                        )
                    for kt in range(KD):
                        nc.tensor.matmul(
                            pu, wu_sb[:, kt], x_view(ci, kt, off, w),
                            start=(kt == 0), stop=(kt == KD - 1),
                        )
                    sl = op.tile([P, w], f32, tag="silu")
                    nc.scalar.activation(sl[:], pg, SiLU)
                    nc.vector.tensor_mul(h_view(ci, ht, off, w), sl[:], pu)

            # ---- DOWN: y = h @ Wd, feature-major [DIM, C] ----
            def load_wd(dt):
                tiles = []
                for s in range(NSET):
                    d = dp.tile([P, HT, P], bf16, tag=f"wd{s}", name=f"wd{s}_{dt}")
                    nc.sync.dma_start(d[:], wdt[s, dt])
                    tiles.append(d)
                return tiles

            d_cache = {0: load_wd(0)}
            for dt in range(DT):
                if dt not in d_cache:
                    d_cache[dt] = load_wd(dt)
                if dt + 1 < DT:
                    d_cache[dt + 1] = load_wd(dt + 1)
                sets = d_cache.pop(dt)
                for ci, (off, w, s) in enumerate(chunks):
                    wd_sb = sets[s]
                    py = ps.tile([P, w], f32, tag="py", bufs=2)
                    for ht in range(HT):
                        nc.tensor.matmul(
                            py, wd_sb[:, ht], h_view(ci, ht, off, w),
                            start=(ht == 0), stop=(ht == HT - 1),
                        )
                    o_sb = op.tile([P, w], f32, tag="o")
                    nc.vector.tensor_copy(o_sb[:], py)
                    nc.sync.dma_start(yt[dt, :, off : off + w], o_sb[:])

    nc.finalize()
    return nc


def _get_kernel(nA, a, nB, b):
    key = (nA, a, nB, b)
    if key not in _KERNEL_CACHE:
        _KERNEL_CACHE[key] = _build_moe_ffn(nA, a, nB, b)
    return _KERNEL_CACHE[key]


def _route(xf, W_gate):
    """Replicate reference routing: top-2 by logit, softmax weights.

    float64 logits: the top-k decision boundary gap is >> f32 rounding
    noise, so this matches the f32 jax reference's selection."""
    logits = xf.astype(np.float64) @ W_gate.astype(np.float64)  # [N, E]
    order = np.argsort(-logits, axis=1, kind="stable")[:, :TOPK]  # [N, 2]
    top = np.take_along_axis(logits, order, axis=1)
    top = top - top.max(axis=1, keepdims=True)
    ew = np.exp(top)
    w = (ew / ew.sum(axis=1, keepdims=True)).astype(np.float32)  # [N, 2]
    return order, w


def _to_bf16(arr):
    import ml_dtypes

    return np.ascontiguousarray(arr.astype(ml_dtypes.bfloat16))


def kernel(x, W_gate, Wg, Wu, Wd):
    from concourse.bass_utils import run_bass_kernel_spmd

    x = np.ascontiguousarray(np.asarray(x, dtype=np.float32))
    W_gate = np.asarray(W_gate, dtype=np.float32)
    Wg = np.asarray(Wg, dtype=np.float32)
    Wu = np.asarray(Wu, dtype=np.float32)
    Wd = np.asarray(Wd, dtype=np.float32)

    B, T, D = x.shape
    xf = x.reshape(-1, D)
    N = xf.shape[0]

    order, w = _route(xf, W_gate)

    ids = []  # per-expert token indices
    wts = []  # per-expert combine weights
    for e in range(E):
        sel = np.nonzero(order == e)
        ids.append(sel[0])
        wts.append(w[sel[0], sel[1]])
    counts = [len(i) for i in ids]

    nA, a, nB, b = _solve_profile(counts)
    acap = nA * a
    C = acap + nB * b
    nc = _get_kernel(nA, a, nB, b)

    # ---- assign overflow (beyond each expert's main slot) to B-slots ----
    # slots[core][j] = (expert, token_ids, token_wts) or None
    slots = [[None] * nB for _ in range(E)]
    free = [(core, j) for j in range(nB) for core in range(E)]
    overflow = []  # (size, expert, ids, wts) slices of width <= b
    for e in range(E):
        rem_i = ids[e][acap:]
        rem_w = wts[e][acap:]
        for s0 in range(0, len(rem_i), b):
            overflow.append((e, rem_i[s0 : s0 + b], rem_w[s0 : s0 + b]))
    assert len(overflow) <= len(free), (counts, nA, a, nB, b)
    for (e, oi, ow), (core, j) in zip(overflow, free):
        slots[core][j] = (e, oi, ow)

    # ---- weight layout transforms (bf16, feature-major tiles) ----
    def wg_tiles(e):
        return Wg[e].reshape(KD, P, HT, P).transpose(2, 1, 0, 3)

    def wu_tiles(e):
        return Wu[e].reshape(KD, P, HT, P).transpose(2, 1, 0, 3)

    def wd_tiles(e):
        return Wd[e].reshape(HT, P, DT, P).transpose(2, 1, 0, 3)

    in_maps = []
    for core in range(E):
        xe = np.zeros((C, DIM), dtype=np.float32)
        cnt_main = min(counts[core], acap)
        xe[:cnt_main] = xf[ids[core][:cnt_main]]
        wg_s = np.zeros((1 + nB, HT, P, KD, P), dtype=np.float32)
        wu_s = np.zeros_like(wg_s)
        wd_s = np.zeros((1 + nB, DT, P, HT, P), dtype=np.float32)
        wg_s[0] = wg_tiles(core)
        wu_s[0] = wu_tiles(core)
        wd_s[0] = wd_tiles(core)
        for j in range(nB):
            if slots[core][j] is None:
                continue
            e, oi, _ow = slots[core][j]
            xe[acap + j * b : acap + j * b + len(oi)] = xf[oi]
            wg_s[1 + j] = wg_tiles(e)
            wu_s[1 + j] = wu_tiles(e)
            wd_s[1 + j] = wd_tiles(e)
        x_t = _to_bf16(xe.T.reshape(KD, P, C).transpose(1, 0, 2))
        in_maps.append(
            {
                "xt": x_t,
                "wgt": _to_bf16(wg_s),
                "wut": _to_bf16(wu_s),
                "wdt": _to_bf16(wd_s),
            }
        )

    res = run_bass_kernel_spmd(nc, in_maps, core_ids=list(range(E)))
    global LAST_RESULTS
    LAST_RESULTS = res

    out = np.zeros((N, D), dtype=np.float32)
    for core in range(E):
        y = res.results[core]["yt"].reshape(DIM, C)  # feature-major
        cnt_main = min(counts[core], acap)
        out[ids[core][:cnt_main]] += (
            wts[core][:cnt_main][:, None] * y[:, :cnt_main].T
        )
        for j in range(nB):
            if slots[core][j] is None:
                continue
            _e, oi, ow = slots[core][j]
            lo = acap + j * b
            out[oi] += ow[:, None] * y[:, lo : lo + len(oi)].T
    return out.reshape(B, T, D)
